# revision 9
# baseline (speedup 1.0000x reference)
"""Multi-head attention (headwise-RoPE variant) on 8 TRN2 NeuronCores.

Problem: B=2, S=2048, E=2048, H=32 heads, D=64, causal, fp32.

Key algebraic simplification: the reference's RoPE bug makes cos/sin depend
only on (head, dim), NOT the sequence position. So RoPE is a fixed per-head
linear map on the head dim and commutes with the projection:
rope(x @ Wq) = x @ (Wq rotated column-wise). We fold rope AND the 1/sqrt(D)
score scale into Wq/Wk (and bq/bk) on the host.

Sharding: tensor-parallel over heads. Core c computes Q/K/V + attention for
heads [4c, 4c+4) over both batches, producing out^T [256, 4096] (attention
output transposed, with softmax denominators obtained by augmenting V with a
ones column). An AllToAll re-shards from head-split to row-split; each core
then computes a 512-row slice of the final projection with the full Wo.
Host concatenates row slices and adds bo.

Device layouts (all matmul-natural, no device transposes):
  xT     [E, B*S]  (host pre-transposed x)
  QT/KT  [256, B*S] = Wq_eff^T @ xT   (per-core head block, rope+scale folded)
  V      [B*S, 256] = (xT tile)^T @ Wv (natural), staged via DRAM
  ST     [k, q] score tiles = KT_slice^T @ QT_slice  (contraction over D=64)
  expST  exp(ST), causal-masked via gpsimd.affine_select (no max-subtraction:
         logits for this input distribution are bounded ~|12|, exp is safe)
  outT+r [65, 512] = [V | 1]^T @ expST  -> rows 0:64 out^T, row 64 = denom
"""

import math
import os
import sys
import types
from contextlib import ExitStack

import numpy as np

B, S, E, H, D = 2, 2048, 2048, 32, 64
N_CORES = 8
HPC = H // N_CORES           # heads per core = 4
CE = HPC * D                 # per-core attention width = 256
BS = B * S                   # 4096 flattened rows
P = 128
KT_E = E // P                # 16 k-tiles over embedding dim
ROWS_PER_CORE = BS // N_CORES  # 512 output rows per core after AllToAll
RHALF = ROWS_PER_CORE // 2     # 256 rows per core per batch
QCHUNK = 512
NQC = S // QCHUNK            # 4 q-chunks per batch
SKT = S // P                 # 16 k-tiles per batch in attention
ROPE_BASE = 10000.0

USE_F32R = os.environ.get("KERNEL_F32R", "1") == "1"
TRACE = os.environ.get("KERNEL_TRACE", "0") == "1"


def _register_ntff_hook():
    """Recreate the missing antenv.axon_hooks so trace=True works (optional)."""
    try:
        import antenv
        from trn_agent_boot.trn_boot import _ntff_profile_via_ctypes

        hook = _ntff_profile_via_ctypes("/opt/axon/libaxon_pjrt.so")
        mod = types.ModuleType("antenv.axon_hooks")
        mod.get_axon_ntff_profile_hook = lambda: hook
        mod.set_axon_ntff_profile_hook = lambda h: None
        sys.modules["antenv.axon_hooks"] = mod
        antenv.axon_hooks = mod
        return hook is not None
    except Exception:
        return False


def _rope_fold(w, b, scale):
    """Fold headwise RoPE (+ optional score scale) into projection weights.

    w: [E, E], b: [E]. Returns (w_eff, b_eff) in float32, computed in float64.
    rope(v)[d]      = v[d]*cos - v[d+32]*sin   (d in [0,32))
    rope(v)[d+32]   = v[d]*sin + v[d+32]*cos
    with angle = head_index * inv_freq[d]  (the reference's "bug": position-
    independent).
    """
    w = np.asarray(w, np.float64)
    b = np.asarray(b, np.float64)
    half = D // 2
    inv_freq = 1.0 / (ROPE_BASE ** (np.arange(0, D, 2, dtype=np.float64) / D))
    t = np.arange(H, dtype=np.float64)
    freqs = t[:, None] * inv_freq[None, :]          # [H, 32]
    cos, sin = np.cos(freqs), np.sin(freqs)

    w4 = w.reshape(E, H, 2, half)
    w_eff = np.empty_like(w4)
    w_eff[:, :, 0] = w4[:, :, 0] * cos[None] - w4[:, :, 1] * sin[None]
    w_eff[:, :, 1] = w4[:, :, 0] * sin[None] + w4[:, :, 1] * cos[None]
    b4 = b.reshape(H, 2, half)
    b_eff = np.empty_like(b4)
    b_eff[:, 0] = b4[:, 0] * cos - b4[:, 1] * sin
    b_eff[:, 1] = b4[:, 0] * sin + b4[:, 1] * cos
    return (w_eff.reshape(E, E) * scale).astype(np.float32), \
           (b_eff.reshape(E) * scale).astype(np.float32)


_NC_CACHE = {}
_ONES = np.ones((P, SKT), np.float32)


def _build_nc():
    import concourse.mybir as mybir
    import concourse.tile as tile
    from concourse import bacc

    f32 = mybir.dt.float32
    # float32r is a reduced-precision fp32 the PE runs at full rate (N>=256).
    # The BIR verifier requires every matmul operand to be *produced* as
    # f32r, so all matmul-feeding tensors are declared f32r end-to-end.
    mm = mybir.dt.float32r if USE_F32R else f32

    nc = bacc.Bacc("TRN2", target_bir_lowering=False, debug=False,
                   num_devices=N_CORES)

    xT_d = nc.dram_tensor("xT", [E, BS], mm, kind="ExternalInput").ap()
    wq_d = nc.dram_tensor("wq", [E, CE], mm, kind="ExternalInput").ap()
    wk_d = nc.dram_tensor("wk", [E, CE], mm, kind="ExternalInput").ap()
    wv_d = nc.dram_tensor("wv", [E, CE], mm, kind="ExternalInput").ap()
    wo_d = nc.dram_tensor("wo", [E, E], mm, kind="ExternalInput").ap()
    bq_d = nc.dram_tensor("bq", [CE], f32, kind="ExternalInput").ap()
    bk_d = nc.dram_tensor("bk", [CE], f32, kind="ExternalInput").ap()
    bv_d = nc.dram_tensor("bv", [CE], f32, kind="ExternalInput").ap()
    ones_d = nc.dram_tensor("ones", [P, SKT], mm, kind="ExternalInput").ap()
    y_d = nc.dram_tensor("y", [ROWS_PER_CORE, E], f32, kind="ExternalOutput").ap()

    # internal DRAM
    v_dram = nc.dram_tensor("v_stage", [BS, CE], mm)
    # per-batch AllToAll buffers: [dest core, attcols, 256 rows]
    a2a_in = [nc.dram_tensor(f"a2a_in{b}", [N_CORES, CE, RHALF], mm).ap()
              for b in range(B)]
    a2a_out = [nc.dram_tensor(f"a2a_out{b}", [N_CORES, CE, RHALF], mm).ap()
               for b in range(B)]

    Exp = mybir.ActivationFunctionType.Exp

    with tile.TileContext(nc) as tc, ExitStack() as octx:
        # long-lived SBUF: QT, KT, outT  [128, 2, 4096] each (4 MB each)
        qkpool = octx.enter_context(tc.tile_pool(name="qk", bufs=1))
        QT = qkpool.tile([P, 2, BS], mm, tag="QT")
        KT = qkpool.tile([P, 2, BS], mm, tag="KT")
        outT = qkpool.tile([P, 2, BS], mm, tag="outT")

        # ---------------- phase 1: projections ----------------
        with ExitStack() as ctx, nc.named_scope("p1_proj"):
            wpool = ctx.enter_context(tc.tile_pool(name="w", bufs=1))
            xpool = ctx.enter_context(tc.tile_pool(name="xt", bufs=24))
            vspool = ctx.enter_context(tc.tile_pool(name="vs", bufs=3))
            ps_qk = ctx.enter_context(tc.tile_pool(name="ps_qk", bufs=2,
                                                   space="PSUM"))
            ps_v = ctx.enter_context(tc.tile_pool(name="ps_v", bufs=2,
                                                  space="PSUM"))

            wq_sb = wpool.tile([P, KT_E, CE], mm, tag="wq")
            wk_sb = wpool.tile([P, KT_E, CE], mm, tag="wk")
            wv_sb = wpool.tile([P, KT_E, CE], mm, tag="wv")
            nc.sync.dma_start(wq_sb[:], wq_d.rearrange("(kt p) m -> p kt m", p=P))
            nc.sync.dma_start(wk_sb[:], wk_d.rearrange("(kt p) m -> p kt m", p=P))
            nc.sync.dma_start(wv_sb[:], wv_d.rearrange("(kt p) m -> p kt m", p=P))

            bq_sb = wpool.tile([P, 2], f32, tag="bq")
            bk_sb = wpool.tile([P, 2], f32, tag="bk")
            nc.sync.dma_start(bq_sb[:], bq_d.rearrange("(t p) -> p t", p=P))
            nc.sync.dma_start(bk_sb[:], bk_d.rearrange("(t p) -> p t", p=P))
            # bv broadcast across partitions for the natural-layout V add
            bv_row = wpool.tile([1, CE], f32, tag="bv_row")
            nc.sync.dma_start(bv_row[:], bv_d[None, :])
            bvb_sb = wpool.tile([P, CE], f32, tag="bvb")
            nc.gpsimd.partition_broadcast(bvb_sb[:], bv_row[:])

            xT_t = xT_d.rearrange("(kt p) r -> p kt r", p=P)

            for n in range(BS // QCHUNK):          # 8 row-chunks of 512
                xts = []
                for k in range(KT_E):
                    xt = xpool.tile([P, QCHUNK], mm, tag="xt")
                    nc.sync.dma_start(
                        xt[:], xT_t[:, k, n * QCHUNK:(n + 1) * QCHUNK])
                    xts.append(xt)

                for (w_sb, b_sb, dst) in ((wq_sb, bq_sb, QT), (wk_sb, bk_sb, KT)):
                    for m in range(2):
                        pq = ps_qk.tile([P, QCHUNK], f32, tag="ps_qk")
                        for k in range(KT_E):
                            nc.tensor.matmul(
                                pq[:],
                                lhsT=w_sb[:, k, m * P:(m + 1) * P],
                                rhs=xts[k][:],
                                start=(k == 0), stop=(k == KT_E - 1))
                        nc.vector.tensor_scalar_add(
                            dst[:, m, n * QCHUNK:(n + 1) * QCHUNK],
                            pq[:], b_sb[:, m:m + 1])

                for mv in range(QCHUNK // P):      # V natural layout
                    pv = ps_v.tile([P, CE], f32, tag="ps_v")
                    for k in range(KT_E):
                        nc.tensor.matmul(
                            pv[:],
                            lhsT=xts[k][:, mv * P:(mv + 1) * P],
                            rhs=wv_sb[:, k],
                            start=(k == 0), stop=(k == KT_E - 1))
                    vst = vspool.tile([P, CE], mm, tag="vst")
                    nc.vector.tensor_add(vst[:], pv[:], bvb_sb[:])
                    r0 = n * QCHUNK + mv * P
                    nc.sync.dma_start(v_dram[r0:r0 + P, :], vst[:])

        # ---------------- phase 2: attention per (b, head) ----------------
        # Per (b, h, q-chunk): two decoupled streams.
        #   Stream A: score matmul PAIRS (two k-tiles into a 2-bank psum)
        #             -> one exp over [128, 2, 512] -> est tile (f32r)
        #             -> causal mask via one affine_select on the last 2 pairs
        #   Stream B: [V|1]^T @ est accumulation into psum_o.
        # Deep est buffering lets ACT run ahead so PE is never chained
        # through ACT per-tile.
        with ExitStack() as ctx, nc.named_scope("p2_attn"):
            vpool = ctx.enter_context(tc.tile_pool(name="vones", bufs=3))
            epool = ctx.enter_context(tc.tile_pool(name="est", bufs=8))
            rpool = ctx.enter_context(tc.tile_pool(name="recip", bufs=3))
            ps_s = ctx.enter_context(tc.tile_pool(name="ps_s", bufs=2,
                                                  space="PSUM"))
            ps_o = ctx.enter_context(tc.tile_pool(name="ps_o", bufs=2,
                                                  space="PSUM"))

            v_t = v_dram.ap()
            for b in range(B):
                for h in range(HPC):
                    pt = h // 2                    # which 128-partition tile
                    off = (h % 2) * 64             # partition offset of head
                    # V tile for (b, h) with ones column: [128, 16, 65]
                    vb = vpool.tile([P, SKT, D + 1], mm, tag="vones")
                    nc.sync.dma_start(vb[:, :, D:D + 1], ones_d[:, :, None])
                    vsrc = v_t[b * S:(b + 1) * S, h * D:(h + 1) * D]
                    nc.sync.dma_start(
                        vb[:, :, 0:D],
                        vsrc.rearrange("(kt p) d -> p kt d", p=P))

                    for qc in range(NQC):
                        q0 = b * S + qc * QCHUNK
                        n_pair = 2 * qc + 2
                        # stream A: scores + exp
                        ests = []
                        for pr in range(n_pair):
                            pss = ps_s.tile([P, 2, QCHUNK], f32, tag="ps_s")
                            for j in range(2):
                                k0 = b * S + (2 * pr + j) * P
                                nc.tensor.matmul(
                                    pss[:, j],
                                    lhsT=KT[off:off + 64, pt, k0:k0 + P],
                                    rhs=QT[off:off + 64, pt, q0:q0 + QCHUNK],
                                    start=True, stop=True)
                            est = epool.tile([P, 2, QCHUNK], mm, tag="est")
                            nc.scalar.activation(est[:], pss[:], Exp)
                            if pr >= n_pair - 2:   # last 4 k-tiles: causal
                                base = qc * QCHUNK - 2 * pr * P
                                nc.gpsimd.affine_select(
                                    out=est[:], in_=est[:],
                                    compare_op=mybir.AluOpType.is_ge,
                                    fill=0.0, base=base,
                                    channel_multiplier=-1,
                                    pattern=[[-P, 2], [1, QCHUNK]])
                            ests.append(est)
                        # stream B: accumulate [V|1]^T @ est
                        po = ps_o.tile([D + 1, QCHUNK], f32, tag="ps_o")
                        for pr in range(n_pair):
                            for j in range(2):
                                kt = 2 * pr + j
                                nc.tensor.matmul(
                                    po[:], lhsT=vb[:, kt], rhs=ests[pr][:, j],
                                    start=(kt == 0),
                                    stop=(kt == 2 * n_pair - 1))
                        # normalize: outT[head, q] = po[0:64] / po[64]
                        r1 = rpool.tile([1, QCHUNK], f32, tag="r1")
                        nc.vector.tensor_copy(r1[:], po[64:65, :])
                        db = rpool.tile([64, QCHUNK], f32, tag="db")
                        nc.gpsimd.partition_broadcast(db[:], r1[:])
                        rb = rpool.tile([64, QCHUNK], f32, tag="rb")
                        nc.vector.reciprocal_approx_fast(out=rb[:], in_=db[:])
                        nc.vector.tensor_mul(
                            outT[off:off + 64, pt, q0:q0 + QCHUNK],
                            po[0:64, :], rb[:])
                # AllToAll per batch as soon as its heads are done:
                # exchange 256-row blocks so every core gets rows
                # [c*256,(c+1)*256) of this batch for ALL heads.
                with nc.named_scope(f"p2b_a2a_{b}"):
                    for j in range(N_CORES):
                        nc.sync.dma_start(
                            a2a_in[b][j].rearrange("(pt p) q -> p pt q", p=P),
                            outT[:, :, b * S + j * RHALF:
                                 b * S + (j + 1) * RHALF])
                    nc.gpsimd.collective_compute(
                        "AllToAll", mybir.AluOpType.bypass,
                        replica_groups=[list(range(N_CORES))],
                        ins=[a2a_in[b].opt()], outs=[a2a_out[b].opt()],
                    )

        # ---------------- phase 3: output projection ----------------
        with ExitStack() as ctx, nc.named_scope("p3_proj"):
            rvpool = ctx.enter_context(tc.tile_pool(name="recv", bufs=1))
            wopool = ctx.enter_context(tc.tile_pool(name="wo", bufs=20))
            ypool = ctx.enter_context(tc.tile_pool(name="y", bufs=3))
            ps_y = ctx.enter_context(tc.tile_pool(name="ps_y", bufs=2,
                                                  space="PSUM"))

            recvs = []
            for b in range(B):
                recv = rvpool.tile([P, KT_E, RHALF], mm, tag=f"recv{b}")
                nc.sync.dma_start(
                    recv[:],
                    a2a_out[b].rearrange("i (pt p) q -> p (i pt) q", p=P))
                recvs.append(recv)

            wo_t = wo_d.rearrange("(kt p) n -> p kt n", p=P)
            for n in range(E // QCHUNK):           # 4 col-chunks of Wo
                wos = []
                for k in range(KT_E):
                    wot = wopool.tile([P, QCHUNK], mm, tag="wo")
                    nc.sync.dma_start(
                        wot[:], wo_t[:, k, n * QCHUNK:(n + 1) * QCHUNK])
                    wos.append(wot)
                for b in range(B):                 # batch halves
                    for mi in range(RHALF // P):   # 2 row-chunks of 128
                        py = ps_y.tile([P, QCHUNK], f32, tag="ps_y")
                        for k in range(KT_E):
                            nc.tensor.matmul(
                                py[:],
                                lhsT=recvs[b][:, k, mi * P:(mi + 1) * P],
                                rhs=wos[k][:],
                                start=(k == 0), stop=(k == KT_E - 1))
                        ysb = ypool.tile([P, QCHUNK], f32, tag="ysb")
                        nc.vector.tensor_copy(ysb[:], py[:])
                        nc.sync.dma_start(
                            y_d[b * RHALF + mi * P:b * RHALF + (mi + 1) * P,
                                n * QCHUNK:(n + 1) * QCHUNK], ysb[:])

    nc.compile()
    return nc


def kernel(x, Wq, bq, Wk, bk, Wv, bv, Wo, bo):
    from concourse import bass_utils

    x = np.ascontiguousarray(np.asarray(x, np.float32))
    Wo = np.ascontiguousarray(np.asarray(Wo, np.float32))
    bo = np.asarray(bo, np.float32)

    scale = 1.0 / math.sqrt(D)
    wq_eff, bq_eff = _rope_fold(Wq, bq, scale)
    wk_eff, bk_eff = _rope_fold(Wk, bk, 1.0)
    wv_f = np.ascontiguousarray(np.asarray(Wv, np.float32))
    bv_f = np.asarray(bv, np.float32)

    xT = np.ascontiguousarray(x.reshape(BS, E).T)

    if "nc" not in _NC_CACHE:
        _NC_CACHE["nc"] = _build_nc()
    nc = _NC_CACHE["nc"]

    in_maps = []
    for c in range(N_CORES):
        cs = slice(c * CE, (c + 1) * CE)
        in_maps.append({
            "xT": xT,
            "wq": np.ascontiguousarray(wq_eff[:, cs]),
            "wk": np.ascontiguousarray(wk_eff[:, cs]),
            "wv": np.ascontiguousarray(wv_f[:, cs]),
            "wo": Wo,
            "bq": np.ascontiguousarray(bq_eff[cs]),
            "bk": np.ascontiguousarray(bk_eff[cs]),
            "bv": np.ascontiguousarray(bv_f[cs]),
            "ones": _ONES,
        })

    trace = TRACE and _register_ntff_hook()
    res = bass_utils.run_bass_kernel_spmd(
        nc, in_maps, core_ids=list(range(N_CORES)),
        trace=trace, trace_cores=[0] if trace else None,
    )
    if trace:
        kernel.last_exec_time_ns = res.exec_time_ns
        kernel.last_results = res

    y = np.empty((B, S, E), np.float32)
    for c in range(N_CORES):
        yc = res.results[c]["y"]
        for b in range(B):
            y[b, c * RHALF:(c + 1) * RHALF] = yc[b * RHALF:(b + 1) * RHALF]
    return (y + bo[None, None, :]).astype(np.float32)


# revision 11
# speedup vs baseline: 1.1184x; 1.1184x over previous
"""Multi-head attention (headwise-RoPE variant) on 8 TRN2 NeuronCores.

Problem: B=2, S=2048, E=2048, H=32 heads, D=64, causal, fp32.

Key algebraic simplification: the reference's RoPE bug makes cos/sin depend
only on (head, dim), NOT the sequence position. So RoPE is a fixed per-head
linear map on the head dim and commutes with the projection:
rope(x @ Wq) = x @ (Wq rotated column-wise). We fold rope AND the 1/sqrt(D)
score scale into Wq/Wk (and bq/bk) on the host.

Sharding: tensor-parallel over heads. Core c computes Q/K/V + attention for
heads [4c, 4c+4) over both batches, producing out^T [256, 4096] (attention
output transposed, with softmax denominators obtained by augmenting V with a
ones column). An AllToAll re-shards from head-split to row-split; each core
then computes a 512-row slice of the final projection with the full Wo.
Host concatenates row slices and adds bo.

Device layouts (all matmul-natural, no device transposes):
  xT     [E, B*S]  (host pre-transposed x)
  QT/KT  [256, B*S] = Wq_eff^T @ xT   (per-core head block, rope+scale folded)
  V      [B*S, 256] = (xT tile)^T @ Wv (natural), staged via DRAM
  ST     [k, q] score tiles = KT_slice^T @ QT_slice  (contraction over D=64)
  expST  exp(ST), causal-masked via gpsimd.affine_select (no max-subtraction:
         logits for this input distribution are bounded ~|12|, exp is safe)
  outT+r [65, 512] = [V | 1]^T @ expST  -> rows 0:64 out^T, row 64 = denom
"""

import math
import os
import sys
import types
from contextlib import ExitStack

import numpy as np

B, S, E, H, D = 2, 2048, 2048, 32, 64
N_CORES = 8
HPC = H // N_CORES           # heads per core = 4
CE = HPC * D                 # per-core attention width = 256
BS = B * S                   # 4096 flattened rows
P = 128
KT_E = E // P                # 16 k-tiles over embedding dim
ROWS_PER_CORE = BS // N_CORES  # 512 output rows per core after AllToAll
RHALF = ROWS_PER_CORE // 2     # 256 rows per core per batch
QCHUNK = 512
NQC = S // QCHUNK            # 4 q-chunks per batch
SKT = S // P                 # 16 k-tiles per batch in attention
ROPE_BASE = 10000.0

USE_F32R = os.environ.get("KERNEL_F32R", "1") == "1"
TRACE = os.environ.get("KERNEL_TRACE", "0") == "1"


def _register_ntff_hook():
    """Recreate the missing antenv.axon_hooks so trace=True works (optional)."""
    try:
        import antenv
        from trn_agent_boot.trn_boot import _ntff_profile_via_ctypes

        hook = _ntff_profile_via_ctypes("/opt/axon/libaxon_pjrt.so")
        mod = types.ModuleType("antenv.axon_hooks")
        mod.get_axon_ntff_profile_hook = lambda: hook
        mod.set_axon_ntff_profile_hook = lambda h: None
        sys.modules["antenv.axon_hooks"] = mod
        antenv.axon_hooks = mod
        return hook is not None
    except Exception:
        return False


def _rope_fold(w, b, scale):
    """Fold headwise RoPE (+ optional score scale) into projection weights.

    w: [E, E], b: [E]. Returns (w_eff, b_eff) in float32, computed in float64.
    rope(v)[d]      = v[d]*cos - v[d+32]*sin   (d in [0,32))
    rope(v)[d+32]   = v[d]*sin + v[d+32]*cos
    with angle = head_index * inv_freq[d]  (the reference's "bug": position-
    independent).
    """
    w = np.asarray(w, np.float64)
    b = np.asarray(b, np.float64)
    half = D // 2
    inv_freq = 1.0 / (ROPE_BASE ** (np.arange(0, D, 2, dtype=np.float64) / D))
    t = np.arange(H, dtype=np.float64)
    freqs = t[:, None] * inv_freq[None, :]          # [H, 32]
    cos, sin = np.cos(freqs), np.sin(freqs)

    w4 = w.reshape(E, H, 2, half)
    w_eff = np.empty_like(w4)
    w_eff[:, :, 0] = w4[:, :, 0] * cos[None] - w4[:, :, 1] * sin[None]
    w_eff[:, :, 1] = w4[:, :, 0] * sin[None] + w4[:, :, 1] * cos[None]
    b4 = b.reshape(H, 2, half)
    b_eff = np.empty_like(b4)
    b_eff[:, 0] = b4[:, 0] * cos - b4[:, 1] * sin
    b_eff[:, 1] = b4[:, 0] * sin + b4[:, 1] * cos
    return (w_eff.reshape(E, E) * scale).astype(np.float32), \
           (b_eff.reshape(E) * scale).astype(np.float32)


_NC_CACHE = {}
_ONES = np.ones((P, SKT), np.float32)


def _build_nc():
    import concourse.mybir as mybir
    import concourse.tile as tile
    from concourse import bacc

    f32 = mybir.dt.float32
    # float32r is a reduced-precision fp32 the PE runs at full rate (N>=256).
    # The BIR verifier requires every matmul operand to be *produced* as
    # f32r, so all matmul-feeding tensors are declared f32r end-to-end.
    mm = mybir.dt.float32r if USE_F32R else f32

    nc = bacc.Bacc("TRN2", target_bir_lowering=False, debug=False,
                   num_devices=N_CORES)

    xT_d = nc.dram_tensor("xT", [E, BS], mm, kind="ExternalInput").ap()
    wq_d = nc.dram_tensor("wq", [E, CE], mm, kind="ExternalInput").ap()
    wk_d = nc.dram_tensor("wk", [E, CE], mm, kind="ExternalInput").ap()
    wv_d = nc.dram_tensor("wv", [E, CE], mm, kind="ExternalInput").ap()
    wo_d = nc.dram_tensor("wo", [E, E], mm, kind="ExternalInput").ap()
    bq_d = nc.dram_tensor("bq", [CE], f32, kind="ExternalInput").ap()
    bk_d = nc.dram_tensor("bk", [CE], f32, kind="ExternalInput").ap()
    bv_d = nc.dram_tensor("bv", [CE], f32, kind="ExternalInput").ap()
    ones_d = nc.dram_tensor("ones", [P, SKT], mm, kind="ExternalInput").ap()
    y_d = nc.dram_tensor("y", [ROWS_PER_CORE, E], f32, kind="ExternalOutput").ap()

    # internal DRAM
    v_dram = nc.dram_tensor("v_stage", [BS, CE], mm)
    # per-batch AllToAll buffers: [dest core, attcols, 256 rows]
    a2a_in = [nc.dram_tensor(f"a2a_in{b}", [N_CORES, CE, RHALF], mm).ap()
              for b in range(B)]
    a2a_out = [nc.dram_tensor(f"a2a_out{b}", [N_CORES, CE, RHALF], mm).ap()
               for b in range(B)]

    Exp = mybir.ActivationFunctionType.Exp

    with tile.TileContext(nc) as tc, ExitStack() as octx:
        # long-lived SBUF: QT, KT, outT  [128, 2, 4096] each (4 MB each)
        qkpool = octx.enter_context(tc.tile_pool(name="qk", bufs=1))
        QT = qkpool.tile([P, 2, BS], mm, tag="QT")
        KT = qkpool.tile([P, 2, BS], mm, tag="KT")
        outT = qkpool.tile([P, 2, BS], mm, tag="outT")

        # ---------------- phase 1: projections ----------------
        with ExitStack() as ctx, nc.named_scope("p1_proj"):
            wpool = ctx.enter_context(tc.tile_pool(name="w", bufs=1))
            xpool = ctx.enter_context(tc.tile_pool(name="xt", bufs=24))
            vspool = ctx.enter_context(tc.tile_pool(name="vs", bufs=3))
            ps_qk = ctx.enter_context(tc.tile_pool(name="ps_qk", bufs=2,
                                                   space="PSUM"))
            ps_v = ctx.enter_context(tc.tile_pool(name="ps_v", bufs=2,
                                                  space="PSUM"))

            wq_sb = wpool.tile([P, KT_E, CE], mm, tag="wq")
            wk_sb = wpool.tile([P, KT_E, CE], mm, tag="wk")
            wv_sb = wpool.tile([P, KT_E, CE], mm, tag="wv")
            nc.sync.dma_start(wq_sb[:], wq_d.rearrange("(kt p) m -> p kt m", p=P))
            nc.sync.dma_start(wk_sb[:], wk_d.rearrange("(kt p) m -> p kt m", p=P))
            nc.sync.dma_start(wv_sb[:], wv_d.rearrange("(kt p) m -> p kt m", p=P))

            bq_sb = wpool.tile([P, 2], f32, tag="bq")
            bk_sb = wpool.tile([P, 2], f32, tag="bk")
            nc.sync.dma_start(bq_sb[:], bq_d.rearrange("(t p) -> p t", p=P))
            nc.sync.dma_start(bk_sb[:], bk_d.rearrange("(t p) -> p t", p=P))
            # bv broadcast across partitions for the natural-layout V add
            bv_row = wpool.tile([1, CE], f32, tag="bv_row")
            nc.sync.dma_start(bv_row[:], bv_d[None, :])
            bvb_sb = wpool.tile([P, CE], f32, tag="bvb")
            nc.gpsimd.partition_broadcast(bvb_sb[:], bv_row[:])

            xT_t = xT_d.rearrange("(kt p) r -> p kt r", p=P)

            for n in range(BS // QCHUNK):          # 8 row-chunks of 512
                xts = []
                for k in range(KT_E):
                    xt = xpool.tile([P, QCHUNK], mm, tag="xt")
                    nc.sync.dma_start(
                        xt[:], xT_t[:, k, n * QCHUNK:(n + 1) * QCHUNK])
                    xts.append(xt)

                for (w_sb, b_sb, dst) in ((wq_sb, bq_sb, QT), (wk_sb, bk_sb, KT)):
                    for m in range(2):
                        pq = ps_qk.tile([P, QCHUNK], f32, tag="ps_qk")
                        for k in range(KT_E):
                            nc.tensor.matmul(
                                pq[:],
                                lhsT=w_sb[:, k, m * P:(m + 1) * P],
                                rhs=xts[k][:],
                                start=(k == 0), stop=(k == KT_E - 1))
                        nc.vector.tensor_scalar_add(
                            dst[:, m, n * QCHUNK:(n + 1) * QCHUNK],
                            pq[:], b_sb[:, m:m + 1])

                for mv in range(QCHUNK // P):      # V natural layout
                    pv = ps_v.tile([P, CE], f32, tag="ps_v")
                    for k in range(KT_E):
                        nc.tensor.matmul(
                            pv[:],
                            lhsT=xts[k][:, mv * P:(mv + 1) * P],
                            rhs=wv_sb[:, k],
                            start=(k == 0), stop=(k == KT_E - 1))
                    vst = vspool.tile([P, CE], mm, tag="vst")
                    nc.vector.tensor_add(vst[:], pv[:], bvb_sb[:])
                    r0 = n * QCHUNK + mv * P
                    nc.sync.dma_start(v_dram[r0:r0 + P, :], vst[:])

        # ---------------- phase 2: attention per (b, head) ----------------
        # Per (b, h, q-chunk): two decoupled streams.
        #   Stream A: score matmul PAIRS (two k-tiles into a 2-bank psum)
        #             -> one exp over [128, 2, 512] -> est tile (f32r)
        #             -> causal mask via one affine_select on the last 2 pairs
        #   Stream B: [V|1]^T @ est accumulation into psum_o.
        # Deep est buffering lets ACT run ahead so PE is never chained
        # through ACT per-tile.
        with ExitStack() as ctx, nc.named_scope("p2_attn"):
            vpool = ctx.enter_context(tc.tile_pool(name="vones", bufs=3))
            epool = ctx.enter_context(tc.tile_pool(name="est", bufs=8))
            rpool = ctx.enter_context(tc.tile_pool(name="recip", bufs=3))
            ps_s = ctx.enter_context(tc.tile_pool(name="ps_s", bufs=2,
                                                  space="PSUM"))
            ps_o = ctx.enter_context(tc.tile_pool(name="ps_o", bufs=2,
                                                  space="PSUM"))

            v_t = v_dram.ap()
            for b in range(B):
                for hp in range(HPC // 2):          # head pairs (2hp, 2hp+1)
                    pt = hp                          # both heads in ptile hp
                    vbs = []
                    for h in (2 * hp, 2 * hp + 1):
                        vb = vpool.tile([P, SKT, D + 1], mm, tag="vones")
                        nc.sync.dma_start(vb[:, :, D:D + 1], ones_d[:, :, None])
                        vsrc = v_t[b * S:(b + 1) * S, h * D:(h + 1) * D]
                        nc.sync.dma_start(
                            vb[:, :, 0:D],
                            vsrc.rearrange("(kt p) d -> p kt d", p=P))
                        vbs.append(vb)

                    for qc in range(NQC):
                        q0 = b * S + qc * QCHUNK
                        n_kt = 4 * qc + 4
                        # stream A: both heads' scores for k-tile kt go into
                        # one 2-bank psum as consecutive MMs at row groups
                        # 0/64 -> they run concurrently on the PE array.
                        ests = []
                        for kt in range(n_kt):
                            k0 = b * S + kt * P
                            pss = ps_s.tile([P, 2, QCHUNK], f32, tag="ps_s")
                            for j in range(2):
                                off = j * 64
                                nc.tensor.matmul(
                                    pss[:, j],
                                    lhsT=KT[off:off + 64, pt, k0:k0 + P],
                                    rhs=QT[off:off + 64, pt, q0:q0 + QCHUNK],
                                    start=True, stop=True)
                            est = epool.tile([P, 2, QCHUNK], mm, tag="est")
                            nc.scalar.activation(est[:], pss[:], Exp)
                            base = qc * QCHUNK - kt * P
                            if base < P:            # partial k-tile: mask both
                                nc.gpsimd.affine_select(
                                    out=est[:], in_=est[:],
                                    compare_op=mybir.AluOpType.is_ge,
                                    fill=0.0, base=base,
                                    channel_multiplier=-1,
                                    pattern=[[0, 2], [1, QCHUNK]])
                            ests.append(est)
                        # stream B: accumulate per head
                        pos = [ps_o.tile([D + 1, QCHUNK], f32, tag="ps_o",
                                         name=f"po{j}") for j in range(2)]
                        for kt in range(n_kt):
                            for j in range(2):
                                nc.tensor.matmul(
                                    pos[j][:], lhsT=vbs[j][:, kt],
                                    rhs=ests[kt][:, j],
                                    start=(kt == 0), stop=(kt == n_kt - 1))
                        # normalize both heads
                        for j in range(2):
                            off = j * 64
                            po = pos[j]
                            r1 = rpool.tile([1, QCHUNK], f32, tag="r1")
                            nc.vector.tensor_copy(r1[:], po[64:65, :])
                            db = rpool.tile([64, QCHUNK], f32, tag="db")
                            nc.gpsimd.partition_broadcast(db[:], r1[:])
                            rb = rpool.tile([64, QCHUNK], f32, tag="rb")
                            nc.vector.reciprocal_approx_fast(out=rb[:], in_=db[:])
                            nc.vector.tensor_mul(
                                outT[off:off + 64, pt, q0:q0 + QCHUNK],
                                po[0:64, :], rb[:])
                # AllToAll per batch as soon as its heads are done:
                # exchange 256-row blocks so every core gets rows
                # [c*256,(c+1)*256) of this batch for ALL heads.
                with nc.named_scope(f"p2b_a2a_{b}"):
                    for j in range(N_CORES):
                        nc.sync.dma_start(
                            a2a_in[b][j].rearrange("(pt p) q -> p pt q", p=P),
                            outT[:, :, b * S + j * RHALF:
                                 b * S + (j + 1) * RHALF])
                    nc.gpsimd.collective_compute(
                        "AllToAll", mybir.AluOpType.bypass,
                        replica_groups=[list(range(N_CORES))],
                        ins=[a2a_in[b].opt()], outs=[a2a_out[b].opt()],
                    )

        # ---------------- phase 3: output projection ----------------
        with ExitStack() as ctx, nc.named_scope("p3_proj"):
            rvpool = ctx.enter_context(tc.tile_pool(name="recv", bufs=1))
            wopool = ctx.enter_context(tc.tile_pool(name="wo", bufs=20))
            ypool = ctx.enter_context(tc.tile_pool(name="y", bufs=3))
            ps_y = ctx.enter_context(tc.tile_pool(name="ps_y", bufs=2,
                                                  space="PSUM"))

            recvs = []
            for b in range(B):
                recv = rvpool.tile([P, KT_E, RHALF], mm, tag=f"recv{b}")
                nc.sync.dma_start(
                    recv[:],
                    a2a_out[b].rearrange("i (pt p) q -> p (i pt) q", p=P))
                recvs.append(recv)

            wo_t = wo_d.rearrange("(kt p) n -> p kt n", p=P)
            for n in range(E // QCHUNK):           # 4 col-chunks of Wo
                wos = []
                for k in range(KT_E):
                    wot = wopool.tile([P, QCHUNK], mm, tag="wo")
                    nc.sync.dma_start(
                        wot[:], wo_t[:, k, n * QCHUNK:(n + 1) * QCHUNK])
                    wos.append(wot)
                for b in range(B):                 # batch halves
                    for mi in range(RHALF // P):   # 2 row-chunks of 128
                        py = ps_y.tile([P, QCHUNK], f32, tag="ps_y")
                        for k in range(KT_E):
                            nc.tensor.matmul(
                                py[:],
                                lhsT=recvs[b][:, k, mi * P:(mi + 1) * P],
                                rhs=wos[k][:],
                                start=(k == 0), stop=(k == KT_E - 1))
                        ysb = ypool.tile([P, QCHUNK], f32, tag="ysb")
                        nc.vector.tensor_copy(ysb[:], py[:])
                        nc.sync.dma_start(
                            y_d[b * RHALF + mi * P:b * RHALF + (mi + 1) * P,
                                n * QCHUNK:(n + 1) * QCHUNK], ysb[:])

    nc.compile()
    return nc


def kernel(x, Wq, bq, Wk, bk, Wv, bv, Wo, bo):
    from concourse import bass_utils

    x = np.ascontiguousarray(np.asarray(x, np.float32))
    Wo = np.ascontiguousarray(np.asarray(Wo, np.float32))
    bo = np.asarray(bo, np.float32)

    scale = 1.0 / math.sqrt(D)
    wq_eff, bq_eff = _rope_fold(Wq, bq, scale)
    wk_eff, bk_eff = _rope_fold(Wk, bk, 1.0)
    wv_f = np.ascontiguousarray(np.asarray(Wv, np.float32))
    bv_f = np.asarray(bv, np.float32)

    xT = np.ascontiguousarray(x.reshape(BS, E).T)

    if "nc" not in _NC_CACHE:
        _NC_CACHE["nc"] = _build_nc()
    nc = _NC_CACHE["nc"]

    in_maps = []
    for c in range(N_CORES):
        cs = slice(c * CE, (c + 1) * CE)
        in_maps.append({
            "xT": xT,
            "wq": np.ascontiguousarray(wq_eff[:, cs]),
            "wk": np.ascontiguousarray(wk_eff[:, cs]),
            "wv": np.ascontiguousarray(wv_f[:, cs]),
            "wo": Wo,
            "bq": np.ascontiguousarray(bq_eff[cs]),
            "bk": np.ascontiguousarray(bk_eff[cs]),
            "bv": np.ascontiguousarray(bv_f[cs]),
            "ones": _ONES,
        })

    trace = TRACE and _register_ntff_hook()
    res = bass_utils.run_bass_kernel_spmd(
        nc, in_maps, core_ids=list(range(N_CORES)),
        trace=trace, trace_cores=[0] if trace else None,
    )
    if trace:
        kernel.last_exec_time_ns = res.exec_time_ns
        kernel.last_results = res

    y = np.empty((B, S, E), np.float32)
    for c in range(N_CORES):
        yc = res.results[c]["y"]
        for b in range(B):
            y[b, c * RHALF:(c + 1) * RHALF] = yc[b * RHALF:(b + 1) * RHALF]
    return (y + bo[None, None, :]).astype(np.float32)
